# revision 46
# baseline (speedup 1.0000x reference)
"""Causal multi-head attention on 8 trn2 NeuronCores.

Problem: B=2, S=2048, D=1024, H=16 heads, HD=64. fp32 in/out.

Sharding: 8 cores = 2 (batch) x 4 (head groups of 4 heads).
Each core computes, for its batch b and head group g:
  Q^T,K^T [256, 2048] (dg on partitions, seq on free) = W^T-slice @ x
  V       [2048, 4*(64+1)]  (natural, a ones column per head)
  per 512-wide q chunk, per head-pair: for each k tile j:
    S^T[k,q] both heads as a ROW-TILED CONCURRENT matmul pair (K=64 each,
    tile_position rows 0-63 / 64-127, ~1.9x PE throughput measured) into
    one [128, 1024] PSUM tile;
    P = exp(S^T/8 - 4) on ACT (fp16 out; the -4 offset prevents fp16
    overflow and cancels exactly in the softmax ratio);
    causal: diagonal k-tiles narrowed to valid q columns, plus a [128,128]
    triangle mask on the diagonal block (split DVE/gpsimd, one per head,
    so neither strict-FIFO queue delays PV);
    PV accumulated over j with V_aug stationary (m=65; row 64 = softmax
    denominator), software-pipelined 4 j-steps behind QK so the ACT exp
    and mask latency never stall the PE FIFO.
  Normalize: one DVE copy frees the PV psum bank (shortest WAR chain for
  the next head-pair); reciprocal + gpsimd partition_broadcast + multiply
  into ctx^T (fp16) then run off the critical path on SBUF data.
  O_partial = ctx^T.T @ Wo_rows [2048, 1024] (fp16 out, ACT HWDGE queue).
Emission interleaves next-chunk projections (and trailing Wo tiles) into
the attention j-loops so projection ACT/DVE work never bunches up at
chunk seams. Engine balance: PE matmuls; ACT exp + QK bias-add + V
copies; DVE masks(h0)/normalize; gpsimd masks(h1)/broadcast.
Host: sums the 4 head-group partials per batch and adds bo + bv @ Wo.

All matmul operands fp16 (1 cycle/row PE rate, halved DMA + SBUF);
accumulation is always fp32 PSUM.
Measured: rel err 7.1e-4; ~171-186 us/iter on HW across sessions
(baseline was ~264 us; depth-3 PV pipeline + Wo copies on ACT beat the
depth-2 variant by ~22 us in a within-process A/B). Engine-occupancy model (TimelineSim): PE 114 us
busy, ACT 92, DVE 46, Pool 31; the HW-vs-sim gap is ~70 ns/matmul
issue+LDWEIGHTS overhead (measured via microbenchmarks) plus the For_i
back-edge all-engine barrier + input-DMA refill (~13 us/iter).
"""

import sys

if "/opt/trn_rl_repo" not in sys.path:
    sys.path.insert(0, "/opt/trn_rl_repo")

import numpy as np

import concourse.bacc as bacc
import concourse.bass as bass
import concourse.mybir as mybir
import concourse.tile as tile
from concourse.bass_utils import run_bass_kernel_spmd

B, S, D, H = 2, 2048, 1024, 16
HD = D // H  # 64
N_CORES = 8
HEADS_PER_CORE = H // 4  # 4
DG = HEADS_PER_CORE * HD  # 256 head dims per core
P = 128
CHUNK = 512  # q chunk width
N_KT = S // P  # 16 k tiles
N_CH = S // CHUNK  # 4 q chunks
F32 = mybir.dt.float32
F16 = mybir.dt.float16
EXP_BIAS = -4.0  # exp(s/8 - 4): fp16-overflow guard, cancels in softmax

_CACHE = {}


def build_kernel(mm_dt="f16in", unroll=1, ablate=()):
    """Build + compile the per-core SPMD program. unroll>1 wraps the body
    in a hardware loop (for pure device timing measurements)."""
    nc = bacc.Bacc("TRN2", target_bir_lowering=False, debug=False)
    xT_d = nc.dram_tensor("xT", [D, S], F16, kind="ExternalInput")
    wq_d = nc.dram_tensor("wq", [D, DG], F16, kind="ExternalInput")
    wk_d = nc.dram_tensor("wk", [D, DG], F16, kind="ExternalInput")
    wv_d = nc.dram_tensor("wv", [D, DG], F16, kind="ExternalInput")
    wo_d = nc.dram_tensor("wo", [DG, D], F16, kind="ExternalInput")
    bq_d = nc.dram_tensor("bq", [DG, 1], F32, kind="ExternalInput")
    bk_d = nc.dram_tensor("bk", [DG, 1], F32, kind="ExternalInput")
    o_d = nc.dram_tensor("o", [S, D], F16, kind="ExternalOutput")

    NDT = D // P  # 8 contraction tiles over D
    NMT = DG // P  # 2 m-tiles over the core's head dims (= head pairs)

    with tile.TileContext(nc) as tc:
        _body(tc, nc,
              xT_d, wq_d, wk_d, wv_d, wo_d, bq_d, bk_d, o_d, NDT, NMT,
              ablate, unroll)

    nc.compile()
    return nc


def _body(tc, nc, xT_d, wq_d, wk_d, wv_d, wo_d, bq_d, bk_d, o_d,
          NDT, NMT, ablate=(), unroll=1):
    import contextlib
    ctx = contextlib.ExitStack()
    with ctx:
        const = ctx.enter_context(tc.tile_pool(name="const", bufs=1))
        sbuf = ctx.enter_context(tc.tile_pool(name="sbuf", bufs=1))
        ptile_p = ctx.enter_context(tc.tile_pool(name="ptile", bufs=8))
        den_p = ctx.enter_context(tc.tile_pool(name="den", bufs=6))
        ctxu_p = ctx.enter_context(tc.tile_pool(name="ctxu", bufs=6))
        out_p = ctx.enter_context(tc.tile_pool(name="outp", bufs=3))
        qkv_ps = ctx.enter_context(
            tc.tile_pool(name="qkv_ps", bufs=2, space="PSUM"))
        stp_ps = ctx.enter_context(
            tc.tile_pool(name="stp_ps", bufs=2, space="PSUM"))
        pv_ps = ctx.enter_context(
            tc.tile_pool(name="pv_ps", bufs=2, space="PSUM"))

        # ---- input tiles ------------------------------------------------
        xt = [const.tile([P, S], F16, tag=f"xt{i}", name=f"xt{i}")
              for i in range(NDT)]
        ws = {}
        for name in ("wq", "wk", "wv"):
            ws[name] = [const.tile([P, DG], F16, tag=f"{name}{i}",
                                   name=f"{name}{i}") for i in range(NDT)]
        wo = [const.tile([P, D], F16, tag=f"wo{m}", name=f"wo{m}")
              for m in range(NMT)]
        biases = {(name, m): const.tile([P, 1], F32, tag=f"{name}{m}",
                                        name=f"{name}{m}")
                  for name in ("bq", "bk") for m in range(NMT)}

        def dma_w(name, d):
            for i in range(NDT):
                nc.sync.dma_start(ws[name][i][:],
                                  d.ap()[P * i:P * (i + 1), :])

        def dma_xt(ci):
            csl = slice(CHUNK * ci, CHUNK * (ci + 1))
            for k in range(NDT):
                nc.sync.dma_start(xt[k][:, csl],
                                  xT_d.ap()[P * k:P * (k + 1), csl])

        def emit_in_dma():
            # order: V(0)+QK(0) deps first, then remaining chunks, wo last
            dma_w("wv", wv_d)
            dma_xt(0)
            dma_w("wq", wq_d)
            dma_w("wk", wk_d)
            for (name, m), t in biases.items():
                d = bq_d if name == "bq" else bk_d
                nc.sync.dma_start(t[:], d.ap()[P * m:P * (m + 1), :])
            for ci in range(1, N_CH):
                dma_xt(ci)
            for m in range(NMT):
                nc.sync.dma_start(wo[m][:], wo_d.ap()[P * m:P * (m + 1), :])

        # ---- constants: vaug ones + causal triangle mask ----------------
        ones_f = const.tile([P, HEADS_PER_CORE], F32, tag="ones_f",
                            name="ones_f")
        ones_r = const.tile([P, HEADS_PER_CORE], F16, tag="ones_r",
                            name="ones_r")
        ebias = const.tile([P, 1], F32, tag="ebias", name="ebias")
        m01 = const.tile([P, P], F16, tag="m01", name="m01")
        m01x2 = const.tile([P, 2 * P], F16, tag="m01x2", name="m01x2")

        def emit_consts():
            nc.vector.memset(ones_f[:], 1.0)
            nc.vector.tensor_copy(ones_r[:], ones_f[:])
            nc.vector.memset(ebias[:], EXP_BIAS)
            # m01[r, c] = 1 if c >= r else 0 (causal triangle, q >= key)
            nc.gpsimd.memset(m01[:], 1.0)
            nc.gpsimd.affine_select(
                out=m01[:], in_=m01[:],
                compare_op=mybir.AluOpType.is_ge,
                fill=0.0, base=0, pattern=[[1, P]],
                channel_multiplier=-1)
            nc.vector.tensor_copy(m01x2[:, 0:P], m01[:])
            nc.vector.tensor_copy(m01x2[:, P:2 * P], m01[:])

        # ---- V projection (natural layout + ones cols) ------------------
        # vaug[j]: [128, 4*65]; head h cols h*65..h*65+63 = V, col h*65+64 = 1
        vaug = [sbuf.tile([P, HEADS_PER_CORE * (HD + 1)], F16,
                          tag=f"vaug{j}", name=f"vaug{j}")
                for j in range(N_KT)]

        def v_proj(j):
            ps = qkv_ps.tile([P, CHUNK], F32, tag="proj", name="proj")
            for k in range(NDT):
                nc.tensor.matmul(
                    ps[:, 0:DG],
                    xt[k][:, P * j:P * (j + 1)],
                    ws["wv"][k][:],
                    start=(k == 0), stop=(k == NDT - 1))
            dst = vaug[j][:].rearrange("p (h x) -> p h x", h=HEADS_PER_CORE)
            srcp = ps[:, 0:DG].rearrange("p (h x) -> p h x", h=HEADS_PER_CORE)
            # ACT copy keeps the (busy, strictly-FIFO) DVE off the PV
            # dependency chain
            nc.scalar.activation(dst[:, :, 0:HD], srcp[:, :, :],
                                 mybir.ActivationFunctionType.Copy)
            nc.vector.tensor_copy(
                dst[:, :, HD:HD + 1],
                ones_r[:].rearrange("p (h x) -> p h x", x=1))

        # ---- Q^T / K^T projections (dg on partitions, fp16) -------------
        qt, kt = [], []
        for name, lst in (("wq", qt), ("wk", kt)):
            for m in range(NMT):
                lst.append(sbuf.tile([P, S], F16, tag=f"{name}T{m}",
                                     name=f"{name}T{m}"))

        def qk_unit(ci, m, name):
            lst = qt if name == "wq" else kt
            bname = "bq" if name == "wq" else "bk"
            ps = qkv_ps.tile([P, CHUNK], F32, tag="proj", name="proj")
            for k in range(NDT):
                nc.tensor.matmul(
                    ps[:],
                    ws[name][k][:, P * m:P * (m + 1)],
                    xt[k][:, CHUNK * ci:CHUNK * (ci + 1)],
                    start=(k == 0), stop=(k == NDT - 1))
            # bias-add on ACT: keeps DVE out of the QK^T dep chain
            nc.scalar.activation(
                lst[m][:, CHUNK * ci:CHUNK * (ci + 1)], ps[:],
                mybir.ActivationFunctionType.Identity,
                bias=biases[(bname, m)][:])

        # ---- attention per (chunk, head pair) ---------------------------
        ctxT = [sbuf.tile([P, S], F16, tag=f"ctxT{m}", name=f"ctxT{m}")
                for m in range(NMT)]

        def attention(ci, filler=()):
            """Emit chunk-ci attention; sprinkle `filler` unit closures
            (next-chunk projections / trailing Wo tiles) between j-steps so
            projection ACT/DVE work never bunches up at chunk seams."""
            filler = list(filler)
            if "qkt" in ablate:
                for f in filler:
                    f()
                return
            jmax = 4 * ci + 3
            total_steps = NMT * (jmax + 1)
            step = 0
            emitted = 0

            def tick():
                nonlocal step, emitted
                step += 1
                while emitted < len(filler) * step // total_steps:
                    filler[emitted]()
                    emitted += 1

            qsl = slice(CHUNK * ci, CHUNK * (ci + 1))
            for pair in range(NMT):
                pv = [pv_ps.tile([HD + 1, CHUNK], F32, tag="pv", name="pv")
                      for _ in range(2)]
                p2s = {}

                def nlo_of(j):
                    dd = j - 4 * ci
                    return P * dd if dd >= 0 else 0

                def emit_pv(j):
                    nlo = nlo_of(j)
                    p2 = p2s.pop(j)
                    if "pv" in ablate and j > 0:
                        return
                    for hh in range(2):
                        h = 2 * pair + hh
                        nc.tensor.matmul(
                            pv[hh][:, nlo:CHUNK],
                            vaug[j][:, (HD + 1) * h:(HD + 1) * (h + 1)],
                            p2[:, CHUNK * hh + nlo:CHUNK * (hh + 1)],
                            start=(j == 0), stop=(j == jmax),
                            skip_group_check=True)

                for j in range(jmax + 1):
                    nlo = nlo_of(j)
                    w = CHUNK - nlo
                    # QK^T: both heads as a concurrent row-tiled pair
                    st2 = stp_ps.tile([P, 2 * CHUNK], F32, tag="stp",
                                      name="stp")
                    for hh in range(2):
                        psl = slice(HD * hh, HD * (hh + 1))
                        nc.tensor.matmul(
                            st2[:, CHUNK * hh + nlo:CHUNK * (hh + 1)],
                            kt[pair][psl, P * j:P * (j + 1)],
                            qt[pair][psl, CHUNK * ci + nlo:CHUNK * (ci + 1)],
                            start=True, stop=True)
                    # exp over both heads in one ACT op (3D AP)
                    p2 = ptile_p.tile([P, 2 * CHUNK], F16, tag="p2",
                                      name="p2")
                    src = st2[:].rearrange("p (h q) -> p h q", h=2)
                    dst = p2[:].rearrange("p (h q) -> p h q", h=2)
                    if "exp" in ablate:
                        nc.vector.tensor_copy(dst[:, :, nlo:CHUNK],
                                              src[:, :, nlo:CHUNK])
                    else:
                        nc.scalar.activation(
                            dst[:, :, nlo:CHUNK], src[:, :, nlo:CHUNK],
                            mybir.ActivationFunctionType.Exp,
                            scale=0.125, bias=ebias[:])
                    # causal triangle mask on the diagonal block, split
                    # across DVE (hh0) and Pool (hh1) so neither strict-FIFO
                    # queue delays the dependent PV matmuls
                    if nlo > 0 or j == 4 * ci:
                        if "mask" not in ablate:
                            p3 = p2[:].rearrange(
                                "p (h q) -> p h q", h=2)[:, :, nlo:nlo + P]
                            m3 = m01x2[:].rearrange(
                                "p (h q) -> p h q", h=2)
                            nc.vector.tensor_mul(p3, p3, m3)
                    p2s[j] = p2
                    if j - 4 >= 0:
                        emit_pv(j - 4)
                    tick()
                for j in (jmax - 3, jmax - 2, jmax - 1, jmax):
                    if j >= 0 and j in p2s:
                        emit_pv(j)

                # ---- softmax divide: ctx^T = pv / den -------------------
                # One plain copy frees the PV psum bank (shortest possible
                # WAR chain for the next pair); the reciprocal/broadcast/
                # multiply then run on SBUF data off the critical path.
                if "div" in ablate:
                    for hh in range(2):
                        nc.vector.tensor_copy(
                            ctxT[pair][HD * hh:HD * (hh + 1), qsl],
                            pv[hh][0:HD, :])
                else:
                    for hh in range(2):
                        cu = ctxu_p.tile([HD + 1, CHUNK], F32,
                                         tag=f"cu{hh}", name=f"cu{hh}")
                        nc.scalar.activation(
                            cu[:], pv[hh][:],
                            mybir.ActivationFunctionType.Copy)
                        rden = den_p.tile([1, CHUNK], F16, tag=f"rden{hh}",
                                          name=f"rden{hh}")
                        with nc.allow_low_precision(
                                reason="1/den fits fp16: den in "
                                       "[e^-6, ~1e4], out rel err 5e-4"):
                            nc.vector.reciprocal(rden[:],
                                                 pv[hh][HD:HD + 1, :])
                        rbc = den_p.tile([HD, CHUNK], F16, tag=f"rbc{hh}",
                                         name=f"rbc{hh}")
                        nc.gpsimd.partition_broadcast(rbc[0:HD, :], rden[:])
                        nc.vector.tensor_mul(
                            ctxT[pair][HD * hh:HD * (hh + 1), qsl],
                            cu[0:HD, :], rbc[0:HD, :])

        # ---- Wo projection (fp16 out) -----------------------------------
        def wo_unit(i):
            ot = out_p.tile([P, D], F16, tag="ot", name="ot")
            pse = [qkv_ps.tile([P, CHUNK], F32, tag="proj", name="proj")
                   for _ in range(2)]
            for m in range(NMT):
                for e in range(2):
                    nc.tensor.matmul(
                        pse[e][:],
                        ctxT[m][:, P * i:P * (i + 1)],
                        wo[m][:, CHUNK * e:CHUNK * (e + 1)],
                        start=(m == 0), stop=(m == NMT - 1))
            for e in range(2):
                # ACT copy: keeps the DVE queue free for the causal masks
                # that gate PV in the diagonal-heavy trailing chunk
                nc.scalar.activation(ot[:, CHUNK * e:CHUNK * (e + 1)],
                                     pse[e][:],
                                     mybir.ActivationFunctionType.Copy)
            if "outdma" not in ablate:
                # output rides the ACT HWDGE queue: the SP input queue is
                # in-order, so putting outputs there would block the next
                # loop iteration's input prefetch behind this iteration's
                # compute
                nc.scalar.dma_start(o_d.ap()[P * i:P * (i + 1), :], ot[:])

        def emit_compute():
            emit_consts()
            # prologue: chunk-0 projections
            for j in range(4):
                v_proj(j)
            for m in range(NMT):
                for name in ("wq", "wk"):
                    qk_unit(0, m, name)
            for ci in range(N_CH):
                if ci + 1 < N_CH:
                    cn = ci + 1
                    units = []
                    qs = [(m, name) for m in range(NMT)
                          for name in ("wq", "wk")]
                    for idx in range(4):
                        units.append(
                            lambda j=4 * cn + idx: v_proj(j))
                        m, name = qs[idx]
                        units.append(
                            lambda cn=cn, m=m, name=name:
                            qk_unit(cn, m, name))
                else:
                    # trailing chunk: interleave Wo tiles of chunks 0-2
                    units = [lambda i=i: wo_unit(i) for i in range(12)]
                attention(ci, units)
            for i in range(12 if N_CH > 1 else 0, S // P):
                wo_unit(i)

        if "indma" in ablate and unroll > 1:
            emit_in_dma()
            with tc.For_i(0, unroll, 1):
                emit_compute()
        elif unroll > 1:
            with tc.For_i(0, unroll, 1):
                emit_in_dma()
                emit_compute()
        else:
            emit_in_dma()
            emit_compute()


def _shard_inputs(x, Wq, bq, Wk, bk, Wv, bv, Wo, bo):
    x = np.asarray(x, np.float32)
    in_maps = []
    for core in range(N_CORES):
        b, g = divmod(core, 4)
        ds = slice(DG * g, DG * (g + 1))
        in_maps.append({
            "xT": np.ascontiguousarray(x[b].T).astype(np.float16),
            "wq": np.ascontiguousarray(
                np.asarray(Wq, np.float32)[:, ds]).astype(np.float16),
            "wk": np.ascontiguousarray(
                np.asarray(Wk, np.float32)[:, ds]).astype(np.float16),
            "wv": np.ascontiguousarray(
                np.asarray(Wv, np.float32)[:, ds]).astype(np.float16),
            "wo": np.ascontiguousarray(
                np.asarray(Wo, np.float32)[ds, :]).astype(np.float16),
            "bq": np.asarray(bq, np.float32)[ds].reshape(DG, 1).copy(),
            "bk": np.asarray(bk, np.float32)[ds].reshape(DG, 1).copy(),
        })
    return in_maps


def kernel(x, Wq, bq, Wk, bk, Wv, bv, Wo, bo):
    mm_dt = _CACHE.get("mm_dt", "f16in")
    _CACHE["mm_dt"] = mm_dt
    if "nc" not in _CACHE:
        _CACHE["nc"] = build_kernel(mm_dt)
    nc = _CACHE["nc"]
    in_maps = _shard_inputs(x, Wq, bq, Wk, bk, Wv, bv, Wo, bo)
    res = run_bass_kernel_spmd(
        nc, in_maps, core_ids=list(range(N_CORES)), trace=False)
    out = np.zeros((B, S, D), np.float32)
    for core in range(N_CORES):
        out[core // 4] += np.asarray(res.results[core]["o"], np.float32)
    # exact bias folding: +bo, + bv @ Wo (constant row vector)
    out += (np.asarray(bo, np.float32)
            + np.asarray(bv, np.float32) @ np.asarray(Wo, np.float32))
    return out


# revision 51
# speedup vs baseline: 1.1704x; 1.1704x over previous
"""Causal multi-head attention on 8 trn2 NeuronCores.

Problem: B=2, S=2048, D=1024, H=16 heads, HD=64. fp32 in/out.

Sharding: 8 cores = 2 (batch) x 4 (head groups of 4 heads).
Each core computes, for its batch b and head group g:
  Q^T,K^T [256, 2048] (dg on partitions, seq on free) = W^T-slice @ x
  V       [2048, 4*(64+1)]  (natural, a ones column per head)
  per 512-wide q chunk, per head-pair: for each k tile j:
    S^T[k,q] both heads as a ROW-TILED CONCURRENT matmul pair (K=64 each,
    tile_position rows 0-63 / 64-127, ~1.9x PE throughput measured) into
    one [128, 1024] PSUM tile;
    P = exp(S^T/8 - 4) on ACT (fp16 out; the -4 offset prevents fp16
    overflow and cancels exactly in the softmax ratio);
    causal: diagonal k-tiles narrowed to valid q columns, plus a [128,128]
    triangle mask on the diagonal block (split DVE/gpsimd, one per head,
    so neither strict-FIFO queue delays PV);
    PV accumulated over j with V_aug stationary (m=65; row 64 = softmax
    denominator), software-pipelined 4 j-steps behind QK so the ACT exp
    and mask latency never stall the PE FIFO.
  Normalize: one DVE copy frees the PV psum bank (shortest WAR chain for
  the next head-pair); reciprocal + gpsimd partition_broadcast + multiply
  into ctx^T (fp16) then run off the critical path on SBUF data.
  O_partial = ctx^T.T @ Wo_rows [2048, 1024] (fp16 out, ACT HWDGE queue).
Emission interleaves next-chunk projections (and trailing Wo tiles) into
the attention j-loops so projection ACT/DVE work never bunches up at
chunk seams. Engine balance: PE matmuls; ACT exp + QK bias-add + V
copies; DVE masks(h0)/normalize; gpsimd masks(h1)/broadcast.
Host: sums the 4 head-group partials per batch and adds bo + bv @ Wo.

All matmul operands fp16 (1 cycle/row PE rate, halved DMA + SBUF);
accumulation is always fp32 PSUM.
Measured: rel err 7.1e-4; ~171-186 us/iter on HW across sessions
(baseline was ~264 us; depth-3 PV pipeline + Wo copies on ACT beat the
depth-2 variant by ~22 us in a within-process A/B). Engine-occupancy model (TimelineSim): PE 114 us
busy, ACT 92, DVE 46, Pool 31; the HW-vs-sim gap is ~70 ns/matmul
issue+LDWEIGHTS overhead (measured via microbenchmarks) plus the For_i
back-edge all-engine barrier + input-DMA refill (~13 us/iter).
"""

import sys

if "/opt/trn_rl_repo" not in sys.path:
    sys.path.insert(0, "/opt/trn_rl_repo")

import numpy as np

import concourse.bacc as bacc
import concourse.bass as bass
import concourse.mybir as mybir
import concourse.tile as tile
from concourse.bass_utils import run_bass_kernel_spmd

B, S, D, H = 2, 2048, 1024, 16
HD = D // H  # 64
N_CORES = 8
HEADS_PER_CORE = H // 4  # 4
DG = HEADS_PER_CORE * HD  # 256 head dims per core
P = 128
CHUNK = 512  # q chunk width
N_KT = S // P  # 16 k tiles
N_CH = S // CHUNK  # 4 q chunks
F32 = mybir.dt.float32
F16 = mybir.dt.float16
EXP_BIAS = -4.0  # exp(s/8 - 4): fp16-overflow guard, cancels in softmax

_CACHE = {}


def build_kernel(mm_dt="f16in", unroll=1, ablate=()):
    """Build + compile the per-core SPMD program. unroll>1 wraps the body
    in a hardware loop (for pure device timing measurements)."""
    nc = bacc.Bacc("TRN2", target_bir_lowering=False, debug=False)
    xT_d = nc.dram_tensor("xT", [D, S], F16, kind="ExternalInput")
    wq_d = nc.dram_tensor("wq", [D, DG], F16, kind="ExternalInput")
    wk_d = nc.dram_tensor("wk", [D, DG], F16, kind="ExternalInput")
    wv_d = nc.dram_tensor("wv", [D, DG], F16, kind="ExternalInput")
    wo_d = nc.dram_tensor("wo", [DG, D], F16, kind="ExternalInput")
    bq_d = nc.dram_tensor("bq", [DG, 1], F32, kind="ExternalInput")
    bk_d = nc.dram_tensor("bk", [DG, 1], F32, kind="ExternalInput")
    o_d = nc.dram_tensor("o", [S, D], F16, kind="ExternalOutput")

    NDT = D // P  # 8 contraction tiles over D
    NMT = DG // P  # 2 m-tiles over the core's head dims (= head pairs)

    with tile.TileContext(nc) as tc:
        _body(tc, nc,
              xT_d, wq_d, wk_d, wv_d, wo_d, bq_d, bk_d, o_d, NDT, NMT,
              ablate, unroll)

    nc.compile()
    return nc


def _body(tc, nc, xT_d, wq_d, wk_d, wv_d, wo_d, bq_d, bk_d, o_d,
          NDT, NMT, ablate=(), unroll=1):
    import contextlib
    ctx = contextlib.ExitStack()
    with ctx:
        const = ctx.enter_context(tc.tile_pool(name="const", bufs=1))
        sbuf = ctx.enter_context(tc.tile_pool(name="sbuf", bufs=1))
        ptile_p = ctx.enter_context(tc.tile_pool(name="ptile", bufs=8))
        den_p = ctx.enter_context(tc.tile_pool(name="den", bufs=6))
        ctxu_p = ctx.enter_context(tc.tile_pool(name="ctxu", bufs=6))
        out_p = ctx.enter_context(tc.tile_pool(name="outp", bufs=3))
        qkv_ps = ctx.enter_context(
            tc.tile_pool(name="qkv_ps", bufs=2, space="PSUM"))
        stp_ps = ctx.enter_context(
            tc.tile_pool(name="stp_ps", bufs=2, space="PSUM"))
        pv_ps = ctx.enter_context(
            tc.tile_pool(name="pv_ps", bufs=2, space="PSUM"))

        # ---- input tiles ------------------------------------------------
        xt = [const.tile([P, S], F16, tag=f"xt{i}", name=f"xt{i}")
              for i in range(NDT)]
        ws = {}
        for name in ("wq", "wk", "wv"):
            ws[name] = [const.tile([P, DG], F16, tag=f"{name}{i}",
                                   name=f"{name}{i}") for i in range(NDT)]
        wo = [const.tile([P, D], F16, tag=f"wo{m}", name=f"wo{m}")
              for m in range(NMT)]
        biases = {(name, m): const.tile([P, 1], F32, tag=f"{name}{m}",
                                        name=f"{name}{m}")
                  for name in ("bq", "bk") for m in range(NMT)}

        def dma_w(name, d):
            for i in range(NDT):
                nc.sync.dma_start(ws[name][i][:],
                                  d.ap()[P * i:P * (i + 1), :])

        def dma_xt(ci):
            csl = slice(CHUNK * ci, CHUNK * (ci + 1))
            for k in range(NDT):
                nc.sync.dma_start(xt[k][:, csl],
                                  xT_d.ap()[P * k:P * (k + 1), csl])

        def emit_in_dma():
            # order: V(0)+QK(0) deps first, then remaining chunks, wo last
            dma_w("wv", wv_d)
            dma_xt(0)
            dma_w("wq", wq_d)
            dma_w("wk", wk_d)
            for (name, m), t in biases.items():
                d = bq_d if name == "bq" else bk_d
                nc.sync.dma_start(t[:], d.ap()[P * m:P * (m + 1), :])
            for ci in range(1, N_CH):
                dma_xt(ci)
            for m in range(NMT):
                nc.sync.dma_start(wo[m][:], wo_d.ap()[P * m:P * (m + 1), :])

        # ---- constants: vaug ones + causal triangle mask ----------------
        ones_f = const.tile([P, HEADS_PER_CORE], F32, tag="ones_f",
                            name="ones_f")
        ones_r = const.tile([P, HEADS_PER_CORE], F16, tag="ones_r",
                            name="ones_r")
        ebias = const.tile([P, 1], F32, tag="ebias", name="ebias")
        m01 = const.tile([P, P], F16, tag="m01", name="m01")

        def emit_consts():
            nc.vector.memset(ones_f[:], 1.0)
            nc.vector.tensor_copy(ones_r[:], ones_f[:])
            nc.vector.memset(ebias[:], EXP_BIAS)
            # m01[r, c] = 1 if c >= r else 0 (causal triangle, q >= key)
            nc.gpsimd.memset(m01[:], 1.0)
            nc.gpsimd.affine_select(
                out=m01[:], in_=m01[:],
                compare_op=mybir.AluOpType.is_ge,
                fill=0.0, base=0, pattern=[[1, P]],
                channel_multiplier=-1)

        # ---- V projection (natural layout + ones cols) ------------------
        # vaug[j]: [128, 4*65]; head h cols h*65..h*65+63 = V, col h*65+64 = 1
        vaug = [sbuf.tile([P, HEADS_PER_CORE * (HD + 1)], F16,
                          tag=f"vaug{j}", name=f"vaug{j}")
                for j in range(N_KT)]

        def v_proj(j):
            ps = qkv_ps.tile([P, CHUNK], F32, tag="proj", name="proj")
            for k in range(NDT):
                nc.tensor.matmul(
                    ps[:, 0:DG],
                    xt[k][:, P * j:P * (j + 1)],
                    ws["wv"][k][:],
                    start=(k == 0), stop=(k == NDT - 1))
            dst = vaug[j][:].rearrange("p (h x) -> p h x", h=HEADS_PER_CORE)
            srcp = ps[:, 0:DG].rearrange("p (h x) -> p h x", h=HEADS_PER_CORE)
            # ACT copy keeps the (busy, strictly-FIFO) DVE off the PV
            # dependency chain
            nc.scalar.activation(dst[:, :, 0:HD], srcp[:, :, :],
                                 mybir.ActivationFunctionType.Copy)
            nc.vector.tensor_copy(
                dst[:, :, HD:HD + 1],
                ones_r[:].rearrange("p (h x) -> p h x", x=1))

        # ---- Q^T / K^T projections (dg on partitions, fp16) -------------
        qt, kt = [], []
        for name, lst in (("wq", qt), ("wk", kt)):
            for m in range(NMT):
                lst.append(sbuf.tile([P, S], F16, tag=f"{name}T{m}",
                                     name=f"{name}T{m}"))

        def qk_unit(ci, m, name):
            lst = qt if name == "wq" else kt
            bname = "bq" if name == "wq" else "bk"
            ps = qkv_ps.tile([P, CHUNK], F32, tag="proj", name="proj")
            for k in range(NDT):
                nc.tensor.matmul(
                    ps[:],
                    ws[name][k][:, P * m:P * (m + 1)],
                    xt[k][:, CHUNK * ci:CHUNK * (ci + 1)],
                    start=(k == 0), stop=(k == NDT - 1))
            # bias-add on ACT: keeps DVE out of the QK^T dep chain
            nc.scalar.activation(
                lst[m][:, CHUNK * ci:CHUNK * (ci + 1)], ps[:],
                mybir.ActivationFunctionType.Identity,
                bias=biases[(bname, m)][:])

        # ---- attention per (chunk, head pair) ---------------------------
        ctxT = [sbuf.tile([P, S], F16, tag=f"ctxT{m}", name=f"ctxT{m}")
                for m in range(NMT)]

        pending = []  # deferred per-(pair,hh) normalize closures

        def attention(ci, filler=()):
            """Emit chunk-ci attention; sprinkle `filler` unit closures
            (next-chunk projections / trailing Wo tiles) between j-steps so
            projection ACT/DVE work never bunches up at chunk seams."""
            filler = list(filler)
            if "qkt" in ablate:
                for f in filler:
                    f()
                return
            jmax = 4 * ci + 3
            total_steps = NMT * (jmax + 1)
            step = 0
            emitted = 0

            def tick():
                nonlocal step, emitted
                step += 1
                # deferred divide ops first: they must be emitted before any
                # filler Wo unit that reads the ctx^T columns they write
                if pending:
                    pending.pop(0)()
                while emitted < len(filler) * step // total_steps:
                    filler[emitted]()
                    emitted += 1

            qsl = slice(CHUNK * ci, CHUNK * (ci + 1))
            for pair in range(NMT):
                pv = [pv_ps.tile([HD + 1, CHUNK], F32, tag="pv", name="pv")
                      for _ in range(2)]
                p2s = {}

                def nlo_of(j):
                    dd = j - 4 * ci
                    return P * dd if dd >= 0 else 0

                def emit_pv(j):
                    nlo = nlo_of(j)
                    p2 = p2s.pop(j)
                    if "pv" in ablate and j > 0:
                        return
                    for hh in range(2):
                        h = 2 * pair + hh
                        nc.tensor.matmul(
                            pv[hh][:, nlo:CHUNK],
                            vaug[j][:, (HD + 1) * h:(HD + 1) * (h + 1)],
                            p2[:, CHUNK * hh + nlo:CHUNK * (hh + 1)],
                            start=(j == 0), stop=(j == jmax),
                            skip_group_check=True)

                for j in range(jmax + 1):
                    nlo = nlo_of(j)
                    w = CHUNK - nlo
                    # QK^T: both heads as a concurrent row-tiled pair
                    st2 = stp_ps.tile([P, 2 * CHUNK], F32, tag="stp",
                                      name="stp")
                    for hh in range(2):
                        psl = slice(HD * hh, HD * (hh + 1))
                        nc.tensor.matmul(
                            st2[:, CHUNK * hh + nlo:CHUNK * (hh + 1)],
                            kt[pair][psl, P * j:P * (j + 1)],
                            qt[pair][psl, CHUNK * ci + nlo:CHUNK * (ci + 1)],
                            start=True, stop=True)
                    # exp over both heads in one ACT op (3D AP)
                    p2 = ptile_p.tile([P, 2 * CHUNK], F16, tag="p2",
                                      name="p2")
                    src = st2[:].rearrange("p (h q) -> p h q", h=2)
                    dst = p2[:].rearrange("p (h q) -> p h q", h=2)
                    if "exp" in ablate:
                        nc.vector.tensor_copy(dst[:, :, nlo:CHUNK],
                                              src[:, :, nlo:CHUNK])
                    else:
                        nc.scalar.activation(
                            dst[:, :, nlo:CHUNK], src[:, :, nlo:CHUNK],
                            mybir.ActivationFunctionType.Exp,
                            scale=0.125, bias=ebias[:])
                    # causal triangle mask on the diagonal block, split
                    # across DVE (hh0) and Pool (hh1) so neither strict-FIFO
                    # queue delays the dependent PV matmuls
                    if nlo > 0 or j == 4 * ci:
                        if "mask" not in ablate:
                            blk0 = slice(nlo, nlo + P)
                            nc.vector.tensor_mul(
                                p2[:, blk0], p2[:, blk0], m01[:])
                            blk1 = slice(CHUNK + nlo, CHUNK + nlo + P)
                            nc.gpsimd.affine_select(
                                out=p2[:, blk1], in_=p2[:, blk1],
                                compare_op=mybir.AluOpType.is_ge,
                                fill=0.0, base=0, pattern=[[1, P]],
                                channel_multiplier=-1)
                    p2s[j] = p2
                    if j - 4 >= 0:
                        emit_pv(j - 4)
                    tick()
                for j in (jmax - 3, jmax - 2, jmax - 1, jmax):
                    if j >= 0 and j in p2s:
                        emit_pv(j)

                # ---- softmax divide: ctx^T = pv / den -------------------
                # One plain copy frees the PV psum bank (shortest possible
                # WAR chain for the next pair); the reciprocal/broadcast/
                # multiply then run on SBUF data off the critical path.
                if "div" in ablate:
                    for hh in range(2):
                        nc.vector.tensor_copy(
                            ctxT[pair][HD * hh:HD * (hh + 1), qsl],
                            pv[hh][0:HD, :])
                else:
                    for hh in range(2):
                        cu = ctxu_p.tile([HD + 1, CHUNK], F32,
                                         tag=f"cu{hh}", name=f"cu{hh}")
                        nc.vector.tensor_copy(cu[:], pv[hh][:])

                        def norm(cu=cu, pair=pair, hh=hh, qsl=qsl):
                            # deferred: runs 1-2 ticks into the next pair's
                            # j-loop, when the DVE/Pool queues are drained,
                            # so it never delays the next pair's masks
                            rden = den_p.tile([1, CHUNK], F16,
                                              tag=f"rden{hh}",
                                              name=f"rden{hh}")
                            with nc.allow_low_precision(
                                    reason="1/den fits fp16: den in "
                                           "[e^-6, ~1e4], rel err 5e-4"):
                                nc.vector.reciprocal(rden[:],
                                                     cu[HD:HD + 1, :])
                            rbc = den_p.tile([HD, CHUNK], F16,
                                             tag=f"rbc{hh}",
                                             name=f"rbc{hh}")
                            nc.gpsimd.partition_broadcast(rbc[0:HD, :],
                                                          rden[:])
                            nc.vector.tensor_mul(
                                ctxT[pair][HD * hh:HD * (hh + 1), qsl],
                                cu[0:HD, :], rbc[0:HD, :])

                        pending.append(norm)

        # ---- Wo projection (fp16 out) -----------------------------------
        def wo_unit(i):
            ot = out_p.tile([P, D], F16, tag="ot", name="ot")
            pse = [qkv_ps.tile([P, CHUNK], F32, tag="proj", name="proj")
                   for _ in range(2)]
            for m in range(NMT):
                for e in range(2):
                    nc.tensor.matmul(
                        pse[e][:],
                        ctxT[m][:, P * i:P * (i + 1)],
                        wo[m][:, CHUNK * e:CHUNK * (e + 1)],
                        start=(m == 0), stop=(m == NMT - 1))
            for e in range(2):
                # ACT copy: keeps the DVE queue free for the causal masks
                # that gate PV in the diagonal-heavy trailing chunk
                nc.scalar.activation(ot[:, CHUNK * e:CHUNK * (e + 1)],
                                     pse[e][:],
                                     mybir.ActivationFunctionType.Copy)
            if "outdma" not in ablate:
                # output rides the ACT HWDGE queue: the SP input queue is
                # in-order, so putting outputs there would block the next
                # loop iteration's input prefetch behind this iteration's
                # compute
                nc.scalar.dma_start(o_d.ap()[P * i:P * (i + 1), :], ot[:])

        def emit_compute():
            emit_consts()
            # prologue: chunk-0 projections
            for j in range(4):
                v_proj(j)
            for m in range(NMT):
                for name in ("wq", "wk"):
                    qk_unit(0, m, name)
            for ci in range(N_CH):
                if ci + 1 < N_CH:
                    cn = ci + 1
                    units = []
                    qs = [(m, name) for m in range(NMT)
                          for name in ("wq", "wk")]
                    for idx in range(4):
                        units.append(
                            lambda j=4 * cn + idx: v_proj(j))
                        m, name = qs[idx]
                        units.append(
                            lambda cn=cn, m=m, name=name:
                            qk_unit(cn, m, name))
                else:
                    # trailing chunk: interleave Wo tiles of chunks 0-2
                    units = [lambda i=i: wo_unit(i) for i in range(12)]
                attention(ci, units)
            while pending:  # last pair's normalizes, before the Wo tail
                pending.pop(0)()
            for i in range(12 if N_CH > 1 else 0, S // P):
                wo_unit(i)

        if "indma" in ablate and unroll > 1:
            emit_in_dma()
            with tc.For_i(0, unroll, 1):
                emit_compute()
        elif unroll > 1:
            with tc.For_i(0, unroll, 1):
                emit_in_dma()
                emit_compute()
        else:
            emit_in_dma()
            emit_compute()


def _shard_inputs(x, Wq, bq, Wk, bk, Wv, bv, Wo, bo):
    x = np.asarray(x, np.float32)
    in_maps = []
    for core in range(N_CORES):
        b, g = divmod(core, 4)
        ds = slice(DG * g, DG * (g + 1))
        in_maps.append({
            "xT": np.ascontiguousarray(x[b].T).astype(np.float16),
            "wq": np.ascontiguousarray(
                np.asarray(Wq, np.float32)[:, ds]).astype(np.float16),
            "wk": np.ascontiguousarray(
                np.asarray(Wk, np.float32)[:, ds]).astype(np.float16),
            "wv": np.ascontiguousarray(
                np.asarray(Wv, np.float32)[:, ds]).astype(np.float16),
            "wo": np.ascontiguousarray(
                np.asarray(Wo, np.float32)[ds, :]).astype(np.float16),
            "bq": np.asarray(bq, np.float32)[ds].reshape(DG, 1).copy(),
            "bk": np.asarray(bk, np.float32)[ds].reshape(DG, 1).copy(),
        })
    return in_maps


def kernel(x, Wq, bq, Wk, bk, Wv, bv, Wo, bo):
    mm_dt = _CACHE.get("mm_dt", "f16in")
    _CACHE["mm_dt"] = mm_dt
    if "nc" not in _CACHE:
        _CACHE["nc"] = build_kernel(mm_dt)
    nc = _CACHE["nc"]
    in_maps = _shard_inputs(x, Wq, bq, Wk, bk, Wv, bv, Wo, bo)
    res = run_bass_kernel_spmd(
        nc, in_maps, core_ids=list(range(N_CORES)), trace=False)
    out = np.zeros((B, S, D), np.float32)
    for core in range(N_CORES):
        out[core // 4] += np.asarray(res.results[core]["o"], np.float32)
    # exact bias folding: +bo, + bv @ Wo (constant row vector)
    out += (np.asarray(bo, np.float32)
            + np.asarray(bv, np.float32) @ np.asarray(Wo, np.float32))
    return out
